# revision 1
# baseline (speedup 1.0000x reference)
# Trainium2 Bass kernel for nn_DASSM (DCN-gated selective-scan module).
#
# Sharding: 8 cores = 4 samples x 2 horizontal bands of 64 rows. All stages
# run band-local (convs/DCN use halo rows recomputed per core); the only
# cross-core dependency is the selective-scan carry at the band boundary,
# exchanged with a pair-wise AllReduce and applied as a decay-weighted
# correction (h += cumprod(dA) * h_in).
#
# Layout: channels (128) on partitions, pixels on the free dim.
import contextlib

import numpy as np

import concourse.bacc as bacc
import concourse.mybir as mybir
import concourse.tile as tile
from concourse.bass_utils import run_bass_kernel_spmd

F32 = mybir.dt.float32
F32R = mybir.dt.float32r
BF16 = mybir.dt.bfloat16
AF = mybir.ActivationFunctionType
OP = mybir.AluOpType

B, C, H, W = 4, 128, 128, 128
G, GC = 8, 16
BAND = 64
XH = 3                      # halo rows of x on each side of the band
NRX = BAND + 2 * XH         # 70 rows in x_pad
NRC = BAND + 4              # 68 rows in xc_pad (band +/- 2)
WP = W + 2                  # padded width
NPIX = BAND * W             # 8192 band pixels
EPS = 1e-6
USE_F32R = False


def _mm(nc, out, lhsT, rhs, start=True, stop=True):
    if USE_F32R:
        lhsT = lhsT.bitcast(F32R)
        rhs = rhs.bitcast(F32R)
    nc.tensor.matmul(out, lhsT, rhs, start=start, stop=stop)


def build_program():
    nc = bacc.Bacc("TRN2", target_bir_lowering=False, debug=False, num_devices=8)

    def inp(name, shape):
        return nc.dram_tensor(name, shape, F32, kind="ExternalInput").ap()

    x_pad = inp("x_pad", [C, NRX, WP])
    w_s1 = inp("w_s1", [C, 9 * C])          # fused in_proj*conv2d taps, lhsT layout
    conv2d_b = inp("conv2d_b", [C, 1])
    dw_k = inp("dw_k", [C, 9])
    dw_b = inp("dw_b", [C, 1])
    ln1_g = inp("ln1_g", [C, 1])
    ln1_b = inp("ln1_b", [C, 1])
    off_w_lhsT = inp("off_w_lhsT", [C, 16])
    off_b_col = inp("off_b_col", [16, 1])
    e6 = inp("e6", [16, 6 * 72])            # expand one-hots: (ya,yb,yc,xa,xb,xc)
    e16 = inp("e16", [72, 9 * C])           # m rows -> per-tap channel bcast (K=72)
    sel_b = inp("sel_b", [10, C])           # x_dbl row 8 -> all channels
    sel_c = inp("sel_c", [10, C])           # x_dbl row 9 -> all channels
    x_proj_lhsT = inp("x_proj_lhsT", [C, 10])
    dt_w_lhsT = inp("dt_w_lhsT", [8, C])
    dt_b_col = inp("dt_b_col", [C, 1])
    a_col = inp("a_col", [C, 1])            # -exp(A_logs)
    ds_col = inp("ds_col", [C, 1])
    ones128 = inp("ones128", [C, C])
    out_w_lhsT = inp("out_w_lhsT", [C, C])  # (out_proj_w * ln2_g).T
    wb2_col = inp("wb2_col", [C, 1])        # out_proj_w @ ln2_b
    mask_contrib = inp("mask_contrib", [C, 1])
    mask_use = inp("mask_use", [C, 1])

    out_band = nc.dram_tensor("out", [C, BAND, W], F32, kind="ExternalOutput").ap()

    with tile.TileContext(nc) as tc:
        est = contextlib.ExitStack()
        sing = est.enter_context(tc.tile_pool(name="sing", bufs=1))

        def load(ap_dram, shape, tagname, dt=F32):
            t = sing.tile(shape, dt, tag=tagname)
            if dt is F32:
                nc.sync.dma_start(out=t[:], in_=ap_dram)
            else:
                nc.gpsimd.dma_start(out=t[:], in_=ap_dram)  # SWDGE casts
            return t

        s_ws1 = load(w_s1, [C, 9 * C], "s_ws1", BF16)
        s_c2b = load(conv2d_b, [C, 1], "s_c2b")
        s_dwk = load(dw_k, [C, 9], "s_dwk")
        s_dwb = load(dw_b, [C, 1], "s_dwb")
        s_l1g = load(ln1_g, [C, 1], "s_l1g")
        s_l1b = load(ln1_b, [C, 1], "s_l1b")
        s_offw = load(off_w_lhsT, [C, 16], "s_offw", BF16)
        s_offb = load(off_b_col, [16, 1], "s_offb")
        s_e6 = load(e6, [16, 6 * 72], "s_e6", BF16)
        s_ones16 = sing.tile([16, 512], BF16, tag="s_ones16")
        nc.vector.memset(s_ones16[:], 1.0)
        s_e16 = load(e16, [72, 9 * C], "s_e16", BF16)
        s_selb = load(sel_b, [10, C], "s_selb")
        s_selc = load(sel_c, [10, C], "s_selc")
        s_xpw = load(x_proj_lhsT, [C, 10], "s_xpw")
        s_dtw = load(dt_w_lhsT, [8, C], "s_dtw")
        s_dtb = load(dt_b_col, [C, 1], "s_dtb")
        s_a = load(a_col, [C, 1], "s_a")
        s_ds = load(ds_col, [C, 1], "s_ds")
        s_o128 = load(ones128, [C, C], "s_o128")
        s_o128b = load(ones128, [C, C], "s_o128b", BF16)
        s_outw = load(out_w_lhsT, [C, C], "s_outw")
        s_wb2 = load(wb2_col, [C, 1], "s_wb2")
        s_mc = load(mask_contrib, [C, 1], "s_mc")
        s_mu = load(mask_use, [C, 1], "s_mu")
        s_eps = sing.tile([C, 1], F32, tag="s_eps")
        nc.vector.memset(s_eps[:], EPS)
        s_zero = sing.tile([C, 1], F32, tag="s_zero")
        nc.vector.memset(s_zero[:], 0.0)
        s_one = sing.tile([C, 1], F32, tag="s_one")
        nc.vector.memset(s_one[:], 1.0)

        # ---- pool stack (LIFO): pxd > pxc > poffs > (pxp | px1 | pm) ----
        pxd_cm = tc.tile_pool(name="pxd", bufs=1)
        pxd = pxd_cm.__enter__()
        pxc_cm = tc.tile_pool(name="pxc", bufs=1)
        pxc = pxc_cm.__enter__()
        pmf_cm = tc.tile_pool(name="pmf", bufs=1)
        pmf = pmf_cm.__enter__()
        poffs_cm = tc.tile_pool(name="poffs", bufs=1)
        poffs = poffs_cm.__enter__()
        xc_pad = pxc.tile([C, NRC, WP], F32)
        nc.vector.memset(xc_pad[:], 0.0)

        # ================= stage 1: fused in_proj + conv2d + SiLU ========
        pxp_cm = tc.tile_pool(name="pxp", bufs=1)
        pxp = pxp_cm.__enter__()
        xp = pxp.tile([C, NRX, WP], BF16)
        nc.gpsimd.dma_start(out=xp[:], in_=x_pad)
        with tc.tile_pool(name="ps1", bufs=2, space="PSUM") as ps1:
            for j0 in range(0, NRC, 4):          # 17 chunks of 4 rows
                pt = ps1.tile([C, 4 * W], F32, tag="ps1")
                for ti in range(9):
                    dy, dx = ti // 3, ti % 3
                    rhs = xp[:, j0 + dy:j0 + dy + 4, dx:dx + W]
                    _mm(nc, pt[:], s_ws1[:, ti * C:(ti + 1) * C], rhs,
                        start=(ti == 0), stop=(ti == 8))
                nc.scalar.activation(
                    out=xc_pad[:, j0:j0 + 4, 1:W + 1],
                    in_=pt[:].rearrange("p (a b) -> p a b", a=4),
                    func=AF.Silu, bias=s_c2b[:], scale=1.0)
        pxp_cm.__exit__(None, None, None)

        # ================= stage 2: depthwise conv -> x1 =================
        px1_cm = tc.tile_pool(name="px1", bufs=1)
        px1 = px1_cm.__enter__()
        x1 = px1.tile([C, BAND, W], BF16)
        for ti in range(9):
            dy, dx = ti // 3, ti % 3
            src = xc_pad[:, 1 + dy:1 + dy + BAND, dx:dx + W]
            if ti == 0:
                nc.vector.tensor_scalar(
                    out=x1[:], in0=src, scalar1=s_dwk[:, 0:1], scalar2=s_dwb[:],
                    op0=OP.mult, op1=OP.add)
            else:
                nc.vector.scalar_tensor_tensor(
                    out=x1[:], in0=src, scalar=s_dwk[:, ti:ti + 1], in1=x1[:],
                    op0=OP.mult, op1=OP.add)

        # ============ LN1 (over channels) + GELU + offset proj ===========
        offs = poffs.tile([16, NPIX], BF16)
        LNC = 1024
        with tc.tile_pool(name="ln1t", bufs=1) as lnt, \
                tc.tile_pool(name="ln1p", bufs=1, space="PSUM") as lnp, \
                tc.tile_pool(name="offp", bufs=1, space="PSUM") as offp:
            x1f = x1[:].rearrange("p a b -> p (a b)")
            for c0 in range(0, NPIX, LNC):
                xc1 = x1f[:, c0:c0 + LNC]
                sq = lnt.tile([C, LNC], BF16, tag="sq")
                nc.scalar.activation(out=sq[:], in_=xc1, func=AF.Square,
                                     bias=s_zero[:], scale=1.0)
                pA = lnp.tile([C, LNC], F32, tag="pA")
                pB = lnp.tile([C, LNC], F32, tag="pB")
                for s0 in range(0, LNC, 512):
                    _mm(nc, pA[:, s0:s0 + 512], s_o128b[:], xc1[:, s0:s0 + 512])
                    _mm(nc, pB[:, s0:s0 + 512], s_o128b[:], sq[:, s0:s0 + 512])
                mu = lnt.tile([C, LNC], F32, tag="mu")
                q = lnt.tile([C, LNC], F32, tag="q")
                nc.vector.tensor_scalar_mul(out=mu[:], in0=pA[:], scalar1=1.0 / C)
                nc.vector.tensor_scalar_mul(out=q[:], in0=pB[:], scalar1=1.0 / C)
                tmp = lnt.tile([C, LNC], F32, tag="tmp")
                nc.vector.tensor_tensor(out=tmp[:], in0=mu[:], in1=mu[:], op=OP.mult)
                nc.vector.tensor_tensor(out=q[:], in0=q[:], in1=tmp[:], op=OP.subtract)
                nc.scalar.activation(out=tmp[:], in_=q[:], func=AF.Ln,
                                     bias=s_eps[:], scale=1.0)
                r = lnt.tile([C, LNC], F32, tag="r")
                nc.scalar.activation(out=r[:], in_=tmp[:], func=AF.Exp,
                                     bias=s_zero[:], scale=-0.5)
                nc.vector.tensor_tensor(out=xc1, in0=xc1, in1=mu[:], op=OP.subtract)
                nc.vector.tensor_tensor(out=xc1, in0=xc1, in1=r[:], op=OP.mult)
                nc.vector.tensor_scalar(out=xc1, in0=xc1, scalar1=s_l1g[:],
                                        scalar2=s_l1b[:], op0=OP.mult, op1=OP.add)
                nc.scalar.activation(out=xc1, in_=xc1, func=AF.Gelu,
                                     bias=s_zero[:], scale=1.0)
                po = offp.tile([16, LNC], F32, tag="po")
                for s0 in range(0, LNC, 512):
                    _mm(nc, po[:, s0:s0 + 512], s_offw[:], xc1[:, s0:s0 + 512])
                nc.scalar.activation(out=offs[:, c0:c0 + LNC], in_=po[:],
                                     func=AF.Identity, bias=s_offb[:], scale=1.0)
        px1_cm.__exit__(None, None, None)

        # ================= DCN factors ===================================
        # fct[:, 0, :] = f_-1 (s then s-a); fct[:, 1, :] = f_+1 (w then w-a).
        # f_0 = 1 - f_-1 - f_+1 is folded into the expand one-hots (e6).
        # Partitions 0-7 = x of groups 0-7, 8-15 = y.
        fct = pmf.tile([16, 2, NPIX], BF16)
        f1 = fct[:, 0, :]
        f2 = fct[:, 1, :]
        at = offs[:]            # offs dead after w; reused as a = s*w
        nc.vector.tensor_scalar(out=f1, in0=offs[:], scalar1=0.0,
                                scalar2=0.0, op0=OP.is_lt, op1=OP.add)
        nc.vector.tensor_tensor(out=f2, in0=offs[:], in1=f1, op=OP.add)
        nc.vector.tensor_tensor(out=at, in0=f1, in1=f2, op=OP.mult)
        nc.vector.tensor_tensor(out=f1, in0=f1, in1=at, op=OP.subtract)
        nc.vector.tensor_tensor(out=f2, in0=f2, in1=at, op=OP.subtract)
        poffs_cm.__exit__(None, None, None)

        # ============ DCN apply (m built per chunk, 9-tap stencil) =======
        xd = pxd.tile([C, BAND, W], F32)
        DCH = 2048
        DR = DCH // W  # 16 rows per chunk
        with tc.tile_pool(name="dcnt", bufs=2) as dcnt, \
                tc.tile_pool(name="dcnm", bufs=2) as dcnm, \
                tc.tile_pool(name="dcnp", bufs=1, space="PSUM") as dcnp, \
                tc.tile_pool(name="dcnp2", bufs=2, space="PSUM") as dcnp2:
            for c0 in range(0, NPIX, DCH):
                t0 = c0 // W
                m_ck = dcnm.tile([72, DCH], BF16, tag="m_ck")
                for s0 in range(0, DCH, 512):
                    pFY = dcnp2.tile([72, 512], F32, tag="pFY")
                    pFX = dcnp2.tile([72, 512], F32, tag="pFX")
                    cs = c0 + s0
                    _mm(nc, pFY[:], s_e6[:, 0 * 72:1 * 72], fct[:, 0, cs:cs + 512],
                        start=True, stop=False)
                    _mm(nc, pFY[:], s_e6[:, 1 * 72:2 * 72], fct[:, 1, cs:cs + 512],
                        start=False, stop=False)
                    _mm(nc, pFY[:], s_e6[:, 2 * 72:3 * 72], s_ones16[:],
                        start=False, stop=True)
                    _mm(nc, pFX[:], s_e6[:, 3 * 72:4 * 72], fct[:, 0, cs:cs + 512],
                        start=True, stop=False)
                    _mm(nc, pFX[:], s_e6[:, 4 * 72:5 * 72], fct[:, 1, cs:cs + 512],
                        start=False, stop=False)
                    _mm(nc, pFX[:], s_e6[:, 5 * 72:6 * 72], s_ones16[:],
                        start=False, stop=True)
                    mfy = dcnt.tile([72, 512], BF16, tag="mfy")
                    nc.vector.tensor_copy(out=mfy[:], in_=pFY[:])
                    nc.vector.tensor_tensor(out=m_ck[:, s0:s0 + 512], in0=mfy[:],
                                            in1=pFX[:], op=OP.mult)
                for ti in range(9):
                    dy, dx = ti // 3, ti % 3
                    pMB = dcnp.tile([C, DCH], F32, tag="pMB")
                    for s0 in range(0, DCH, 512):
                        _mm(nc, pMB[:, s0:s0 + 512], s_e16[:, ti * C:(ti + 1) * C],
                            m_ck[:, s0:s0 + 512])
                    src = xc_pad[:, 1 + dy + t0:1 + dy + t0 + DR, dx:dx + W]
                    dst = xd[:, t0:t0 + DR, :]
                    pmb3 = pMB[:].rearrange("p (a b) -> p a b", a=DR)
                    if ti == 0:
                        nc.vector.tensor_tensor(out=dst, in0=src, in1=pmb3, op=OP.mult)
                    else:
                        tmp = dcnt.tile([C, DR, W], F32, tag="dtmp")
                        nc.vector.tensor_tensor(out=tmp[:], in0=src, in1=pmb3, op=OP.mult)
                        nc.vector.tensor_tensor(out=dst, in0=dst, in1=tmp[:], op=OP.add)
        pmf_cm.__exit__(None, None, None)
        pxc_cm.__exit__(None, None, None)

        # ====== x_proj; fused dts/delta/dA/u(dBx) per chunk ==============
        xdf = xd[:].rearrange("p a b -> p (a b)")
        pbig_cm = tc.tile_pool(name="pbig", bufs=1)
        pbig = pbig_cm.__enter__()
        xdbl = pbig.tile([10, NPIX], F32)
        dA = pbig.tile([C, NPIX], F32, tag="dA")
        u = pbig.tile([C, NPIX], F32, tag="u")
        with tc.tile_pool(name="dtt", bufs=2) as dtt, \
                tc.tile_pool(name="pp2", bufs=2, space="PSUM") as pp2:
            for c0 in range(0, NPIX, 512):
                pt = pp2.tile([10, 512], F32, tag="pxdbl")
                _mm(nc, pt[:], s_xpw[:], xdf[:, c0:c0 + 512])
                nc.vector.tensor_copy(out=xdbl[:, c0:c0 + 512], in_=pt[:])
            for c0 in range(0, NPIX, 512):
                pt = pp2.tile([C, 512], F32, tag="pdts")
                _mm(nc, pt[:], s_dtw[:], xdbl[0:8, c0:c0 + 512])
                dch = dtt.tile([C, 512], F32, tag="dch")
                # softplus(z) = ln(1 + exp(z)); z <= ~-1.9 here so exp is safe
                nc.scalar.activation(out=dch[:], in_=pt[:],
                                     func=AF.Exp, bias=s_dtb[:], scale=1.0)
                nc.scalar.activation(out=dch[:], in_=dch[:],
                                     func=AF.Ln, bias=s_one[:], scale=1.0)
                nc.scalar.activation(out=dA[:, c0:c0 + 512], in_=dch[:],
                                     func=AF.Exp, bias=s_zero[:], scale=s_a[:])
                # u = delta * x * B
                nc.vector.tensor_tensor(out=dch[:], in0=dch[:],
                                        in1=xdf[:, c0:c0 + 512], op=OP.mult)
                pb = pp2.tile([C, 512], F32, tag="pb")
                _mm(nc, pb[:], s_selb[:], xdbl[:, c0:c0 + 512])
                nc.vector.tensor_tensor(out=u[:, c0:c0 + 512], in0=dch[:],
                                        in1=pb[:], op=OP.mult)

        # ================= selective scan + carry ========================
        h = pbig.tile([C, NPIX], F32, tag="h")
        nc.vector.tensor_tensor_scan(out=h[:], data0=dA[:], data1=u[:],
                                     initial=0.0, op0=OP.mult, op1=OP.add)
        # exchange h_last within band pairs
        hc = sing.tile([C, 1], F32)
        nc.vector.tensor_tensor(out=hc[:], in0=h[:, NPIX - 1:NPIX], in1=s_mc[:],
                                op=OP.mult)
        with tc.tile_pool(name="dramp", bufs=1, space="DRAM") as dramp:
            cc_in = dramp.tile([C, 1], F32)
            cc_out = dramp.tile([C, 1], F32)
            nc.sync.dma_start(out=cc_in[:], in_=hc[:])
            nc.gpsimd.collective_compute(
                "AllReduce", OP.add,
                replica_groups=[[0, 1], [2, 3], [4, 5], [6, 7]],
                ins=[cc_in[:].opt()], outs=[cc_out[:].opt()])
            h_in = sing.tile([C, 1], F32)
            nc.sync.dma_start(out=h_in[:], in_=cc_out[:])
        nc.vector.tensor_tensor(out=h_in[:], in0=h_in[:], in1=s_mu[:], op=OP.mult)
        # E = cumprod(dA) computed in place over dA; h += E * h_in
        zeros = pbig.tile([C, NPIX], F32, tag="u")
        nc.vector.memset(zeros[:], 0.0)
        nc.vector.tensor_tensor_scan(out=dA[:], data0=dA[:], data1=zeros[:],
                                     initial=1.0, op0=OP.mult, op1=OP.add)
        nc.vector.scalar_tensor_tensor(out=h[:], in0=dA[:], scalar=h_in[:],
                                       in1=h[:], op0=OP.mult, op1=OP.add)

        # ================= y = h*C + Ds*x ================================
        y = pbig.tile([C, NPIX], F32, tag="u")
        with tc.tile_pool(name="pcc", bufs=2, space="PSUM") as pcc:
            for c0 in range(0, NPIX, 512):
                pt = pcc.tile([C, 512], F32, tag="pc")
                _mm(nc, pt[:], s_selc[:], xdbl[:, c0:c0 + 512])
                nc.vector.tensor_tensor(out=y[:, c0:c0 + 512], in0=h[:, c0:c0 + 512],
                                        in1=pt[:], op=OP.mult)
        nc.vector.scalar_tensor_tensor(out=y[:], in0=xdf, scalar=s_ds[:],
                                       in1=y[:], op0=OP.mult, op1=OP.add)

        # ================= LN2 + out_proj ================================
        osb = pbig.tile([C, NPIX], F32, tag="dA")
        LNC2 = 512
        with tc.tile_pool(name="ln2t", bufs=1) as lnt2, \
                tc.tile_pool(name="ln2p", bufs=1, space="PSUM") as lnp2:
            for c0 in range(0, NPIX, LNC2):
                yc = y[:, c0:c0 + LNC2]
                sq = lnt2.tile([C, LNC2], BF16, tag="sq2")
                nc.scalar.activation(out=sq[:], in_=yc, func=AF.Square,
                                     bias=s_zero[:], scale=1.0)
                pA = lnp2.tile([C, LNC2], F32, tag="pA2")
                pB = lnp2.tile([C, LNC2], F32, tag="pB2")
                for s0 in range(0, LNC2, 512):
                    _mm(nc, pA[:, s0:s0 + 512], s_o128[:], yc[:, s0:s0 + 512])
                    _mm(nc, pB[:, s0:s0 + 512], s_o128b[:], sq[:, s0:s0 + 512])
                mu = lnt2.tile([C, LNC2], F32, tag="mu2")
                q = lnt2.tile([C, LNC2], F32, tag="q2")
                nc.vector.tensor_scalar_mul(out=mu[:], in0=pA[:], scalar1=1.0 / C)
                nc.vector.tensor_scalar_mul(out=q[:], in0=pB[:], scalar1=1.0 / C)
                tmp = lnt2.tile([C, LNC2], F32, tag="tmp2")
                nc.vector.tensor_tensor(out=tmp[:], in0=mu[:], in1=mu[:], op=OP.mult)
                nc.vector.tensor_tensor(out=q[:], in0=q[:], in1=tmp[:], op=OP.subtract)
                nc.scalar.activation(out=tmp[:], in_=q[:], func=AF.Ln,
                                     bias=s_eps[:], scale=1.0)
                r = lnt2.tile([C, LNC2], F32, tag="r2")
                nc.scalar.activation(out=r[:], in_=tmp[:], func=AF.Exp,
                                     bias=s_zero[:], scale=-0.5)
                nc.vector.tensor_tensor(out=yc, in0=yc, in1=mu[:], op=OP.subtract)
                nc.vector.tensor_tensor(out=yc, in0=yc, in1=r[:], op=OP.mult)
                pO = lnp2.tile([C, LNC2], F32, tag="pO")
                for s0 in range(0, LNC2, 512):
                    _mm(nc, pO[:, s0:s0 + 512], s_outw[:], yc[:, s0:s0 + 512])
                nc.scalar.activation(out=osb[:, c0:c0 + LNC2], in_=pO[:],
                                     func=AF.Identity, bias=s_wb2[:], scale=1.0)
        nc.sync.dma_start(out=out_band,
                          in_=osb[:].rearrange("p (a b) -> p a b", a=BAND))
        pbig_cm.__exit__(None, None, None)
        pxd_cm.__exit__(None, None, None)
        est.close()
    nc.finalize()
    return nc


_CACHE = {}


def _host_prep(inputs):
    """Build the per-core in_maps from the full inputs."""
    x = inputs["x"].astype(np.float32)
    in_proj_w = inputs["in_proj_w"].astype(np.float32)
    k1 = inputs["conv2d_w"].astype(np.float32)[:, 0]        # (C,3,3)
    w_s1 = np.zeros((C, 9 * C), np.float32)                 # lhsT per tap [c, o]
    for ti in range(9):
        dy, dx = ti // 3, ti % 3
        Wt = in_proj_w * k1[:, dy, dx][:, None]             # (o, c)
        w_s1[:, ti * C:(ti + 1) * C] = Wt.T
    perm = list(range(0, 16, 2)) + list(range(1, 16, 2))
    off_w_p = inputs["off_w"].astype(np.float32)[perm]      # (16, C)
    off_b_p = inputs["off_b"].astype(np.float32)[perm]
    # expand one-hots: m row p = dy*24 + dx*8 + g; fct row k = axis*8 + g
    # FY gets +f_-1 at dy=0(-1), +f_+1 at dy=2(+1), and f_0 = 1 - f_-1 - f_+1
    # at dy=1 via (-1, -1, +1*ones).  Same for FX over dx.
    e6 = np.zeros((16, 6 * 72), np.float32)
    for g in range(8):
        for d in range(3):
            # FY (uses y rows: k = 8 + g)
            e6[8 + g, 0 * 72 + 0 * 24 + d * 8 + g] = 1.0   # f_-1 -> dy=-1
            e6[8 + g, 0 * 72 + 1 * 24 + d * 8 + g] = -1.0  # -f_-1 -> dy=0
            e6[8 + g, 1 * 72 + 2 * 24 + d * 8 + g] = 1.0   # f_+1 -> dy=+1
            e6[8 + g, 1 * 72 + 1 * 24 + d * 8 + g] = -1.0  # -f_+1 -> dy=0
            e6[0 + g, 2 * 72 + 1 * 24 + d * 8 + g] = 1.0   # ones -> dy=0 (y rows0-7? no)
            # FX (uses x rows: k = g)
            e6[0 + g, 3 * 72 + d * 24 + 0 * 8 + g] = 1.0
            e6[0 + g, 3 * 72 + d * 24 + 1 * 8 + g] = -1.0
            e6[0 + g, 4 * 72 + d * 24 + 2 * 8 + g] = 1.0
            e6[0 + g, 4 * 72 + d * 24 + 1 * 8 + g] = -1.0
            e6[8 + g, 5 * 72 + d * 24 + 1 * 8 + g] = 1.0
    e16 = np.zeros((72, 9 * C), np.float32)
    for ti in range(9):
        for c in range(C):
            e16[ti * 8 + c // GC, ti * C + c] = 1.0
    sel_b = np.zeros((10, C), np.float32)
    sel_b[8, :] = 1.0
    sel_c = np.zeros((10, C), np.float32)
    sel_c[9, :] = 1.0
    ln2_g = inputs["out_ln_g"].astype(np.float32)
    ln2_b = inputs["out_ln_b"].astype(np.float32)
    out_w = inputs["out_proj_w"].astype(np.float32)
    shared = dict(
        w_s1=w_s1,
        conv2d_b=inputs["conv2d_b"].astype(np.float32).reshape(C, 1),
        dw_k=np.ascontiguousarray(
            inputs["dw_w"].astype(np.float32)[:, 0].reshape(C, 9)),
        dw_b=inputs["dw_b"].astype(np.float32).reshape(C, 1),
        ln1_g=inputs["dw_ln_g"].astype(np.float32).reshape(C, 1),
        ln1_b=inputs["dw_ln_b"].astype(np.float32).reshape(C, 1),
        off_w_lhsT=np.ascontiguousarray(off_w_p.T),
        off_b_col=off_b_p.reshape(16, 1),
        e6=e6, e16=e16, sel_b=sel_b, sel_c=sel_c,
        x_proj_lhsT=np.ascontiguousarray(inputs["x_proj_w"].astype(np.float32).T),
        dt_w_lhsT=np.ascontiguousarray(inputs["dt_w"].astype(np.float32).T),
        dt_b_col=inputs["dt_b"].astype(np.float32).reshape(C, 1),
        a_col=(-np.exp(inputs["A_logs"].astype(np.float32)[:, 0])).reshape(C, 1),
        ds_col=inputs["Ds"].astype(np.float32).reshape(C, 1),
        ones128=np.ones((C, C), np.float32),
        out_w_lhsT=np.ascontiguousarray((out_w * ln2_g[None, :]).T),
        wb2_col=(out_w @ ln2_b).reshape(C, 1),
    )
    in_maps = []
    for core in range(8):
        b, half = core // 2, core % 2
        r0 = half * BAND
        xp = np.zeros((C, NRX, WP), np.float32)
        lo, hi = r0 - XH, r0 + BAND + XH
        slo, shi = max(lo, 0), min(hi, H)
        xp[:, slo - lo:shi - lo, 1:W + 1] = x[b, :, slo:shi, :]
        im = dict(shared)
        im["x_pad"] = xp
        im["mask_contrib"] = np.full((C, 1), 1.0 - half, np.float32)
        im["mask_use"] = np.full((C, 1), float(half), np.float32)
        in_maps.append(im)
    return in_maps


def kernel(**inputs) -> np.ndarray:
    if "nc" not in _CACHE:
        _CACHE["nc"] = build_program()
    nc = _CACHE["nc"]
    in_maps = _host_prep(inputs)
    res = run_bass_kernel_spmd(nc, in_maps, core_ids=list(range(8)))
    out = np.zeros((B, C, H, W), np.float32)
    for core in range(8):
        b, half = core // 2, core % 2
        out[b, :, half * BAND:(half + 1) * BAND, :] = res.results[core]["out"]
    return out


if __name__ == "__main__":
    import jax
    with jax.default_device(jax.devices("cpu")[0]):
        import reference as R
        inp = {k: np.asarray(v) for k, v in R.setup_inputs().items()}
    got = kernel(**inp)
    ref = np.load("/root/problem/ref_out.npy")
    rel = np.linalg.norm(got - ref) / np.linalg.norm(ref)
    print("Relative error:", rel)

